# revision 32
# baseline (speedup 1.0000x reference)
"""AdaptiveWindowAttention distributed Bass kernel for 8 TRN2 NeuronCores.

Sharding: core c -> batch b = c//4, head group g = c%4 (heads 4g..4g+3).
 - QKV projection: bf16 matmuls (x, w_qkv shipped pre-transposed+bf16),
   f32 PSUM accumulation, n-sequential accumulation groups (low PSUM pressure).
 - RoPE via permutation-matmul (rot = P @ q) + DVE combine -> bf16 q/k.
 - Windowed causal attention: window <= 256, so scores are computed on a
   384-wide band (3 k-tiles per q-tile) against 256-zero-padded K/V. The
   runtime window enters via a host-computed additive mask-bias tensor.
   No max-subtraction: |scores*scale| <= ~20 so exp can't overflow f32.
 - P transposed on PE (3 slices into one PSUM bank), AV + out-projection
   feed from SBUF-resident tiles (no DRAM round-trips).
 - Out-projection partial sums ReduceScatter'ed (add) across each 4-core
   group in 4 row-chunks; host reassembles the full [2,2048,2048] output.
"""
import os
import sys

import numpy as np

for _p in ("/opt/trn_rl_repo",):
    if _p not in sys.path and os.path.isdir(_p):
        sys.path.insert(0, _p)

EMB = 2048
HEADS = 16
HD = 128
S = 2048
B = 2
SCALE = HD ** -0.5
NEG = -1.0e5
NCORES = 8

_CACHE = {}


# ----------------------------------------------------------------- host math
def _host_window(x, w_c1, w_c2):
    xf = x.reshape(B, -1).astype(np.float64)
    var = xf.var(axis=1, ddof=1)
    var_norm = 1.0 / (1.0 + np.exp(-(var * 10.0 - 5.0)))
    x_mean = x.mean(axis=1).astype(np.float64)
    h = x_mean @ w_c1.T.astype(np.float64)
    h = h / (1.0 + np.exp(-h))
    learned = 1.0 / (1.0 + np.exp(-(h @ w_c2.T.astype(np.float64))))[:, 0]
    complexity = (var_norm + learned) / 2.0
    window_f = 64.0 + complexity * (256.0 - 64.0)
    w = int(np.float32(window_f.mean()))
    return max(min(w, S), 64)


def _build_maskbias(window):
    mb = np.empty((128, 3, 384), dtype=np.float32)
    jmin = [256, 128, 0]
    q = np.arange(128)[:, None]
    j = np.arange(384)[None, :]
    rel = q + 256 - j
    for v in range(3):
        keep = (rel >= 0) & (rel < window) & (j >= jmin[v])
        mb[:, v, :] = np.where(keep, 0.0, NEG)
    return mb


def _rope_tables():
    inv_freq = 1.0 / (10000.0 ** (np.arange(0, HD, 2, dtype=np.float32) / HD))
    pos = np.arange(S, dtype=np.float32)
    freqs = np.outer(pos, inv_freq)
    emb = np.concatenate([freqs, freqs], axis=-1)  # [S, 128]
    return (np.ascontiguousarray(np.cos(emb).T).astype(np.float32),
            np.ascontiguousarray(np.sin(emb).T).astype(np.float32))


def _perm_mat():
    P = np.zeros((HD, HD), dtype=np.float32)
    for i in range(HD // 2):
        P[2 * i, 2 * i + 1] = -1.0
        P[2 * i + 1, 2 * i] = 1.0
    return P  # rot = P @ q ; shipped as lhsT = P.T


# ----------------------------------------------------------------- bass build
def _build_nc(single_core=False, phases=3, iters=1):
    import concourse.bass as bass  # noqa: F401
    from concourse import bacc, mybir, tile

    f32 = mybir.dt.float32
    f32r = mybir.dt.float32r
    bf16 = mybir.dt.bfloat16
    AF = mybir.ActivationFunctionType
    ALU = mybir.AluOpType
    AX = mybir.AxisListType

    nc = bacc.Bacc("TRN2", target_bir_lowering=False, debug=False,
                   num_devices=1 if single_core else NCORES)

    xT_d = nc.dram_tensor("xT", [EMB, S], bf16, kind="ExternalInput").ap()
    wqkT_d = nc.dram_tensor("wqkT", [EMB, 1024], bf16, kind="ExternalInput").ap()
    wvT_d = nc.dram_tensor("wvT", [EMB, 512], bf16, kind="ExternalInput").ap()
    woT_d = nc.dram_tensor("woT", [512, EMB], bf16, kind="ExternalInput").ap()
    cosT_d = nc.dram_tensor("cosT", [HD, S], bf16, kind="ExternalInput").ap()
    sinT_d = nc.dram_tensor("sinT", [HD, S], bf16, kind="ExternalInput").ap()
    permT_d = nc.dram_tensor("permT", [HD, HD], f32r, kind="ExternalInput").ap()
    ident_d = nc.dram_tensor("ident", [HD, HD], f32r, kind="ExternalInput").ap()
    mb_d = nc.dram_tensor("mb", [128, 3, 384], f32, kind="ExternalInput").ap()
    out_d = nc.dram_tensor("out", [512, EMB], f32, kind="ExternalOutput").ap()

    xT_r = xT_d.rearrange("(c p) s -> p c s", p=128)        # [128,16,S]
    wqkT_r = wqkT_d.rearrange("(c p) m -> p c m", p=128)    # [128,16,1024]
    wvT_r = wvT_d.rearrange("(c p) m -> p c m", p=128)      # [128,16,512]
    woT_r = woT_d.rearrange("(h p) n -> p h n", p=128)      # [128,4,EMB]

    RG = [[0, 1, 2, 3], [4, 5, 6, 7]]

    with tile.TileContext(nc) as tc:
        from contextlib import ExitStack
        with ExitStack() as ctx:
            resid = ctx.enter_context(tc.tile_pool(name="resid", bufs=1))
            psp = ctx.enter_context(tc.tile_pool(name="ps", bufs=8, space="PSUM"))
            dramp = ctx.enter_context(tc.tile_pool(name="dram", bufs=1, space="DRAM"))

            qT = resid.tile([128, 4, S], bf16, tag="qT")
            kT = resid.tile([128, 4, 256 + S], bf16, tag="kT")
            vP = resid.tile([128, 18, 512], f32r, tag="vP")
            perm_sb = resid.tile([128, 128], f32r, tag="perm")
            ident_sb = resid.tile([128, 128], f32r, tag="ident")
            cs_sb = resid.tile([128, 2, S], bf16, tag="cs")
            mb_sb = resid.tile([128, 3, 384], f32, tag="mb")
            zr = resid.tile([128, 512], f32, tag="zr")

            nc.sync.dma_start(perm_sb[:], permT_d[:])
            nc.sync.dma_start(ident_sb[:], ident_d[:])
            nc.sync.dma_start(cs_sb[:, 0, :], cosT_d[:])
            nc.sync.dma_start(cs_sb[:, 1, :], sinT_d[:])
            nc.sync.dma_start(mb_sb[:], mb_d[:])
            nc.vector.memset(zr[:], 0.0)
            for _h in range(4):
                nc.scalar.copy(kT[:, _h, 0:256], zr[:, 0:256])
            for _c in range(2):
                nc.scalar.copy(vP[:, _c, :], zr[:])

            part_dram = dramp.tile([S, EMB], f32)
            rs_dram = dramp.tile([512, EMB], f32)

            for _it in range(iters):
              # ------------- phase 1: qkv projection + rope + v transpose
              with ExitStack() as p1:
                xp = p1.enter_context(tc.tile_pool(name="xp", bufs=16))
                wqp = p1.enter_context(tc.tile_pool(name="wqp", bufs=3))
                wvp = p1.enter_context(tc.tile_pool(name="wvp", bufs=1))
                tp = p1.enter_context(tc.tile_pool(name="tp", bufs=3))

                xts = []
                for c in range(16):
                    xt = xp.tile([128, S], bf16, tag="xt", name=f"xt{c}")
                    nc.sync.dma_start(xt[:], xT_r[:, c, :])
                    xts.append(xt)

                wv_t = wvp.tile([128, 16, 512], bf16, tag="wv")
                for c in range(16):
                    nc.sync.dma_start(wv_t[:, c, :], wvT_r[:, c, :])

                def v_proj():
                    # V projection: direct [s, d] orientation (lhsT = x chunk)
                    for ss in range(16):
                        psv = psp.tile([128, 512], f32, tag="ps",
                                       name=f"psv_{ss}")
                        for c in range(16):
                            nc.tensor.matmul(
                                psv[:], xts[c][:, ss * 128:(ss + 1) * 128],
                                wv_t[:, c, :], start=(c == 0), stop=(c == 15))
                        nc.scalar.copy(vP[:, ss + 2, :], psv[:])

                v_proj()
                mt_order = []
                for h in range(4):
                    mt_order += [h, 4 + h]
                for mt in mt_order:
                    wq_t = wqp.tile([128, 16, 128], bf16, tag="wq")
                    for c in range(16):
                        nc.sync.dma_start(wq_t[:, c, :],
                                          wqkT_r[:, c, mt * 128:(mt + 1) * 128])
                    for n in range(4):
                        ps = psp.tile([128, 512], f32, tag="ps",
                                      name=f"ps_{mt}_{n}")
                        for c in range(16):
                            nc.tensor.matmul(
                                ps[:], wq_t[:, c, :], xts[c][:, n * 512:(n + 1) * 512],
                                start=(c == 0), stop=(c == 15))
                        if True:
                            h = mt % 4
                            is_q = mt < 4
                            tmp = tp.tile([128, 512], f32r, tag="tmp")
                            nc.scalar.copy(tmp[:], ps[:])
                            rot = psp.tile([128, 512], f32, tag="ps",
                                           name=f"rot_{mt}_{n}")
                            nc.tensor.matmul(rot[:], perm_sb[:], tmp[:],
                                             start=True, stop=True)
                            if is_q:
                                dest = qT[:, h, n * 512:(n + 1) * 512]
                            else:
                                dest = kT[:, h, 256 + n * 512: 256 + (n + 1) * 512]
                            nc.vector.tensor_mul(
                                out=dest, in0=rot[:],
                                in1=cs_sb[:, 1, n * 512:(n + 1) * 512])
                            nc.vector.tensor_mul(
                                out=tmp[:], in0=tmp[:],
                                in1=cs_sb[:, 0, n * 512:(n + 1) * 512])
                            nc.vector.tensor_add(out=dest, in0=dest, in1=tmp[:])

              if phases < 2:
                  ob1 = resid.tile([128, 512], f32, tag="ob1")
                  nc.vector.tensor_copy(ob1[:], qT[:, 0, 0:512])
                  nc.sync.dma_start(out_d[0:128, 0:512], ob1[:])
                  continue

              # ------------- phase 2+3: banded attention, out-proj, RS
              with ExitStack() as p2:
                ptp = p2.enter_context(tc.tile_pool(name="ptp", bufs=3))
                sp = p2.enter_context(tc.tile_pool(name="sp", bufs=10))
                smp = p2.enter_context(tc.tile_pool(name="smp", bufs=8))
                atp = p2.enter_context(tc.tile_pool(name="atp", bufs=1))
                wop = p2.enter_context(tc.tile_pool(name="wop", bufs=1))
                obp = p2.enter_context(tc.tile_pool(name="obp", bufs=4))

                attnT = atp.tile([128, 4, S], bf16, tag="attnT")
                wo_sb = wop.tile([128, 4, EMB], bf16, tag="wo")
                if phases >= 3:
                    for h in range(4):
                        nc.sync.dma_start(wo_sb[:, h, :], woT_r[:, h, :])

                for p in range(8):  # pairs of q-tiles
                    for hw_ in range(1):  # full wave: 4 heads x 2 qt
                        hs = (0, 1, 2, 3)
                        PTs = {}
                        pexps = {}
                        for h in hs:
                            PT = ptp.tile([128, 4, 256], f32r, tag="PT",
                                          name=f"PT_{p}_{h}")
                            nc.vector.tensor_copy(PT[:, 0, 128:256], zr[:, 0:128])
                            nc.vector.tensor_copy(PT[:, 3, 0:128], zr[:, 0:128])
                            PTs[h] = PT
                        # wave 1: all 4 score matmuls
                        for h in hs:
                            for qi in range(2):
                                qt = 2 * p + qi
                                sc_ps = psp.tile([128, 384], f32, tag="ps",
                                                 name=f"sc_{p}_{h}_{qi}")
                                nc.tensor.matmul(
                                    sc_ps[:],
                                    qT[:, h, qt * 128:(qt + 1) * 128],
                                    kT[:, h, qt * 128: qt * 128 + 384],
                                    start=True, stop=True)
                                v_idx = min(qt, 2)
                                sc = sp.tile([128, 384], f32, tag="sc",
                                             name=f"scb_{p}_{h}_{qi}")
                                nc.vector.tensor_add(out=sc[:], in0=sc_ps[:],
                                                     in1=mb_sb[:, v_idx, :])
                                pexp = sp.tile([128, 384], f32r, tag="pexp",
                                               name=f"pexp_{p}_{h}_{qi}")
                                rs = smp.tile([128, 1], f32, tag="rs")
                                nc.scalar.activation(pexp[:], sc[:], AF.Exp,
                                                     bias=0.0, scale=SCALE,
                                                     accum_out=rs[:])
                                rr = smp.tile([128, 1], f32, tag="rr")
                                nc.vector.reciprocal(rr[:], rs[:])
                                nc.vector.tensor_scalar_mul(pexp[:], pexp[:], rr[:])
                                pexps[(h, qi)] = pexp
                        # wave 2: transposes + PT assembly
                        for h in hs:
                            for qi in range(2):
                                pexp = pexps[(h, qi)]
                                pt3 = psp.tile([128, 384], f32r, tag="ps",
                                               name=f"pt3_{p}_{h}_{qi}")
                                for j in range(3):
                                    nc.tensor.transpose(
                                        pt3[:, j * 128:(j + 1) * 128],
                                        pexp[:, j * 128:(j + 1) * 128],
                                        ident_sb[:])
                                nc.scalar.copy(
                                    PTs[h][:, qi:qi + 3, qi * 128:(qi + 1) * 128],
                                    pt3[:].rearrange("p (c q) -> p c q", c=3))
                        # wave 3: AV
                        for h in hs:
                            av = psp.tile([128, 256], f32, tag="ps",
                                          name=f"av_{p}_{h}")
                            for cc in range(4):
                                nc.tensor.matmul(
                                    av[:],
                                    vP[:, 2 * p + cc, h * 128:(h + 1) * 128],
                                    PTs[h][:, cc, :],
                                    start=(cc == 0), stop=(cc == 3))
                            nc.scalar.copy(
                                attnT[:, h, p * 256:(p + 1) * 256], av[:])

                    if phases < 3:
                        continue
                    # out-projection for the two s-tiles of this pair
                    for sti in range(2):
                        st = 2 * p + sti
                        for nt in range(4):
                            po = psp.tile([128, 512], f32, tag="ps",
                                          name=f"po_{st}_{nt}")
                            for h in range(4):
                                nc.tensor.matmul(
                                    po[:], attnT[:, h, st * 128:(st + 1) * 128],
                                    wo_sb[:, h, nt * 512:(nt + 1) * 512],
                                    start=(h == 0), stop=(h == 3))
                            ob = obp.tile([128, 512], f32, tag="ob")
                            nc.scalar.copy(ob[:], po[:])
                            nc.sync.dma_start(
                                part_dram[st * 128:(st + 1) * 128,
                                          nt * 512:(nt + 1) * 512], ob[:])

                    # chunked reduce-scatter once a row-group is complete
                    if phases >= 3 and p % 2 == 1:
                        g = p // 2
                        if single_core:
                            nc.sync.dma_start(
                                rs_dram[g * 128:(g + 1) * 128, :],
                                part_dram[g * 512:g * 512 + 128, :])
                        else:
                            nc.gpsimd.collective_compute(
                                "ReduceScatter", ALU.add, replica_groups=RG,
                                ins=[part_dram[g * 512:(g + 1) * 512, :].opt()],
                                outs=[rs_dram[g * 128:(g + 1) * 128, :].opt()])
                        nc.sync.dma_start(out_d[g * 128:(g + 1) * 128, :],
                                          rs_dram[g * 128:(g + 1) * 128, :])

    nc.compile()
    return nc


def _get_nc():
    if "nc" not in _CACHE:
        _CACHE["nc"] = _build_nc()
    return _CACHE["nc"]


# ----------------------------------------------------------------- entry
def _prepare_in_maps(x, w_qkv, w_out, w_c1, w_c2):
    import ml_dtypes

    x = np.asarray(x, dtype=np.float32)
    w_qkv = np.asarray(w_qkv, dtype=np.float32)
    w_out = np.asarray(w_out, dtype=np.float32)
    w_c1 = np.asarray(w_c1, dtype=np.float32)
    w_c2 = np.asarray(w_c2, dtype=np.float32)

    window = _host_window(x, w_c1, w_c2)
    mb = _build_maskbias(window)
    cosT, sinT = _rope_tables()
    permT = np.ascontiguousarray(_perm_mat().T)  # lhsT for rot matmul
    ident = np.eye(128, dtype=np.float32)
    bf = ml_dtypes.bfloat16

    in_maps = []
    for c in range(NCORES):
        b, g = divmod(c, 4)
        rows = slice(g * 512, g * 512 + 512)
        wq = w_qkv[0 * EMB:1 * EMB][rows]
        wk = w_qkv[1 * EMB:2 * EMB][rows]
        wv = w_qkv[2 * EMB:3 * EMB][rows]
        in_maps.append({
            "xT": np.ascontiguousarray(x[b].T).astype(bf),
            "wqkT": np.ascontiguousarray(np.concatenate([wq, wk], 0).T).astype(bf),
            "wvT": np.ascontiguousarray(wv.T).astype(bf),
            "woT": np.ascontiguousarray(w_out.T[rows]).astype(bf),
            "cosT": cosT.astype(bf), "sinT": sinT.astype(bf),
            "permT": permT, "ident": ident,
            "mb": mb,
        })
    return in_maps


def _assemble(results):
    out = np.empty((B, S, EMB), dtype=np.float32)
    for c in range(NCORES):
        b, j = divmod(c, 4)
        r = results[c]["out"]  # [512, EMB]
        for g in range(4):
            out[b, g * 512 + j * 128: g * 512 + (j + 1) * 128] = \
                r[g * 128:(g + 1) * 128]
    return out


def kernel(x, w_qkv, w_out, w_c1, w_c2):
    from concourse.bass_utils import run_bass_kernel_spmd

    nc = _get_nc()
    in_maps = _prepare_in_maps(x, w_qkv, w_out, w_c1, w_c2)
    res = run_bass_kernel_spmd(nc, in_maps, core_ids=list(range(NCORES)))
    return _assemble(res.results)


# revision 43
# speedup vs baseline: 1.2038x; 1.2038x over previous
"""AdaptiveWindowAttention distributed Bass kernel for 8 TRN2 NeuronCores.

Sharding: core c -> batch b = c//4, head group g = c%4 (heads 4g..4g+3).
 - QKV projection: bf16 matmuls (x, w_qkv shipped pre-transposed+bf16),
   f32 PSUM accumulation, n-sequential accumulation groups (low PSUM pressure).
 - RoPE via permutation-matmul (rot = P @ q) + DVE combine -> bf16 q/k.
 - Windowed causal attention: window <= 256, so scores are computed on a
   384-wide band (3 k-tiles per q-tile) against 256-zero-padded K/V. The
   runtime window enters via a host-computed additive mask-bias tensor.
   No max-subtraction: |scores*scale| <= ~20 so exp can't overflow f32.
 - P transposed on PE (3 slices into one PSUM bank), AV + out-projection
   feed from SBUF-resident tiles (no DRAM round-trips).
 - Out-projection partial sums ReduceScatter'ed (add) across each 4-core
   group in 4 row-chunks; host reassembles the full [2,2048,2048] output.
"""
import os
import sys

import numpy as np

for _p in ("/opt/trn_rl_repo",):
    if _p not in sys.path and os.path.isdir(_p):
        sys.path.insert(0, _p)

EMB = 2048
HEADS = 16
HD = 128
S = 2048
B = 2
SCALE = HD ** -0.5
NEG = -1.0e5
NCORES = 8

_CACHE = {}


# ----------------------------------------------------------------- host math
def _host_window(x, w_c1, w_c2):
    xf = x.reshape(B, -1).astype(np.float64)
    var = xf.var(axis=1, ddof=1)
    var_norm = 1.0 / (1.0 + np.exp(-(var * 10.0 - 5.0)))
    x_mean = x.mean(axis=1).astype(np.float64)
    h = x_mean @ w_c1.T.astype(np.float64)
    h = h / (1.0 + np.exp(-h))
    learned = 1.0 / (1.0 + np.exp(-(h @ w_c2.T.astype(np.float64))))[:, 0]
    complexity = (var_norm + learned) / 2.0
    window_f = 64.0 + complexity * (256.0 - 64.0)
    w = int(np.float32(window_f.mean()))
    return max(min(w, S), 64)


def _build_maskbias(window):
    mb = np.empty((128, 3, 384), dtype=np.float32)
    jmin = [256, 128, 0]
    q = np.arange(128)[:, None]
    j = np.arange(384)[None, :]
    rel = q + 256 - j
    for v in range(3):
        keep = (rel >= 0) & (rel < window) & (j >= jmin[v])
        mb[:, v, :] = np.where(keep, 0.0, NEG)
    return mb


def _rope_tables():
    inv_freq = 1.0 / (10000.0 ** (np.arange(0, HD, 2, dtype=np.float32) / HD))
    pos = np.arange(S, dtype=np.float32)
    freqs = np.outer(pos, inv_freq)
    emb = np.concatenate([freqs, freqs], axis=-1)  # [S, 128]
    return (np.ascontiguousarray(np.cos(emb).T).astype(np.float32),
            np.ascontiguousarray(np.sin(emb).T).astype(np.float32))


def _perm_mat():
    P = np.zeros((HD, HD), dtype=np.float32)
    for i in range(HD // 2):
        P[2 * i, 2 * i + 1] = -1.0
        P[2 * i + 1, 2 * i] = 1.0
    return P  # rot = P @ q ; shipped as lhsT = P.T


# ----------------------------------------------------------------- bass build
def _build_nc(single_core=False, phases=3, iters=1):
    import concourse.bass as bass  # noqa: F401
    from concourse import bacc, mybir, tile

    f32 = mybir.dt.float32
    f32r = mybir.dt.float32r
    bf16 = mybir.dt.bfloat16
    AF = mybir.ActivationFunctionType
    ALU = mybir.AluOpType
    AX = mybir.AxisListType

    nc = bacc.Bacc("TRN2", target_bir_lowering=False, debug=False,
                   num_devices=1 if single_core else NCORES)

    xT_d = nc.dram_tensor("xT", [EMB, S], bf16, kind="ExternalInput").ap()
    wqkT_d = nc.dram_tensor("wqkT", [EMB, 1024], bf16, kind="ExternalInput").ap()
    wvT_d = nc.dram_tensor("wvT", [EMB, 512], bf16, kind="ExternalInput").ap()
    woT_d = nc.dram_tensor("woT", [512, EMB], bf16, kind="ExternalInput").ap()
    cosT_d = nc.dram_tensor("cosT", [HD, S], bf16, kind="ExternalInput").ap()
    sinT_d = nc.dram_tensor("sinT", [HD, S], bf16, kind="ExternalInput").ap()
    permT_d = nc.dram_tensor("permT", [HD, HD], f32r, kind="ExternalInput").ap()
    ident_d = nc.dram_tensor("ident", [HD, HD], f32r, kind="ExternalInput").ap()
    mb_d = nc.dram_tensor("mb", [128, 3, 384], f32, kind="ExternalInput").ap()
    out_d = nc.dram_tensor("out", [512, EMB], f32, kind="ExternalOutput").ap()

    xT_r = xT_d.rearrange("(c p) s -> p c s", p=128)        # [128,16,S]
    wqkT_r = wqkT_d.rearrange("(c p) m -> p c m", p=128)    # [128,16,1024]
    wvT_r = wvT_d.rearrange("(c p) m -> p c m", p=128)      # [128,16,512]
    woT_r = woT_d.rearrange("(h p) n -> p h n", p=128)      # [128,4,EMB]

    RG = [[0, 1, 2, 3], [4, 5, 6, 7]]

    with tile.TileContext(nc) as tc:
        from contextlib import ExitStack
        with ExitStack() as ctx:
            resid = ctx.enter_context(tc.tile_pool(name="resid", bufs=1))
            psp = ctx.enter_context(tc.tile_pool(name="ps", bufs=8, space="PSUM"))
            dramp = ctx.enter_context(tc.tile_pool(name="dram", bufs=1, space="DRAM"))

            qT = resid.tile([128, 4, S], bf16, tag="qT")
            kT = resid.tile([128, 4, 256 + S], bf16, tag="kT")
            vP = resid.tile([128, 18, 512], f32r, tag="vP")
            perm_sb = resid.tile([128, 128], f32r, tag="perm")
            ident_sb = resid.tile([128, 128], f32r, tag="ident")
            cs_sb = resid.tile([128, 2, S], bf16, tag="cs")
            mb_sb = resid.tile([128, 3, 384], f32, tag="mb")
            zr = resid.tile([128, 512], f32, tag="zr")

            nc.vector.memset(zr[:], 0.0)
            for _h in range(4):
                nc.scalar.copy(kT[:, _h, 0:256], zr[:, 0:256])
            for _c in range(2):
                nc.scalar.copy(vP[:, _c, :], zr[:])

            part_dram = dramp.tile([S, EMB], f32)
            rs_dram = dramp.tile([512, EMB], f32)

            for _it in range(iters):
              # ------------- phase 1: qkv projection + rope + v transpose
              with ExitStack() as p1:
                xp = p1.enter_context(tc.tile_pool(name="xp", bufs=16))
                wqp = p1.enter_context(tc.tile_pool(name="wqp", bufs=3))
                wvp = p1.enter_context(tc.tile_pool(name="wvp", bufs=1))
                tp = p1.enter_context(tc.tile_pool(name="tp", bufs=6))

                xts = []
                wv_t = wvp.tile([128, 16, 512], bf16, tag="wv")
                for c in range(16):
                    xt = xp.tile([128, S], bf16, tag="xt", name=f"xt{c}")
                    nc.sync.dma_start(xt[:], xT_r[:, c, :])
                    nc.sync.dma_start(wv_t[:, c, :], wvT_r[:, c, :])
                    xts.append(xt)
                if _it == 0:
                    nc.sync.dma_start(perm_sb[:], permT_d[:])
                    nc.sync.dma_start(ident_sb[:], ident_d[:])
                    nc.sync.dma_start(cs_sb[:, 0, :], cosT_d[:])
                    nc.sync.dma_start(cs_sb[:, 1, :], sinT_d[:])
                    nc.sync.dma_start(mb_sb[:], mb_d[:])

                def v_proj():
                    # V projection: direct [s, d] orientation (lhsT = x chunk)
                    for ss in range(16):
                        psv = psp.tile([128, 512], f32, tag="ps",
                                       name=f"psv_{ss}")
                        for c in range(16):
                            nc.tensor.matmul(
                                psv[:], xts[c][:, ss * 128:(ss + 1) * 128],
                                wv_t[:, c, :], start=(c == 0), stop=(c == 15))
                        nc.scalar.copy(vP[:, ss + 2, :], psv[:])

                v_proj()
                mt_order = []
                for h in range(4):
                    mt_order += [h, 4 + h]
                for mt in mt_order:
                    wq_t = wqp.tile([128, 16, 128], bf16, tag="wq")
                    for c in range(16):
                        nc.sync.dma_start(wq_t[:, c, :],
                                          wqkT_r[:, c, mt * 128:(mt + 1) * 128])
                    h = mt % 4
                    is_q = mt < 4
                    pss = []
                    tmps = []
                    for n in range(4):
                        ps = psp.tile([128, 512], f32, tag="ps",
                                      name=f"ps_{mt}_{n}")
                        for c in range(16):
                            nc.tensor.matmul(
                                ps[:], wq_t[:, c, :], xts[c][:, n * 512:(n + 1) * 512],
                                start=(c == 0), stop=(c == 15))
                        tmp = tp.tile([128, 512], f32r, tag="tmp",
                                      name=f"tmp_{mt}_{n}")
                        nc.scalar.copy(tmp[:], ps[:])
                        pss.append(ps)
                        tmps.append(tmp)
                    for n in range(4):
                        tmp = tmps[n]
                        rot = psp.tile([128, 512], f32, tag="ps",
                                       name=f"rot_{mt}_{n}")
                        nc.tensor.matmul(rot[:], perm_sb[:], tmp[:],
                                         start=True, stop=True)
                        if is_q:
                            dest = qT[:, h, n * 512:(n + 1) * 512]
                        else:
                            dest = kT[:, h, 256 + n * 512: 256 + (n + 1) * 512]
                        nc.vector.tensor_mul(
                            out=dest, in0=rot[:],
                            in1=cs_sb[:, 1, n * 512:(n + 1) * 512])
                        nc.vector.tensor_mul(
                            out=tmp[:], in0=tmp[:],
                            in1=cs_sb[:, 0, n * 512:(n + 1) * 512])
                        nc.vector.tensor_add(out=dest, in0=dest, in1=tmp[:])

              if phases < 2:
                  ob1 = resid.tile([128, 512], f32, tag="ob1")
                  nc.vector.tensor_copy(ob1[:], qT[:, 0, 0:512])
                  nc.sync.dma_start(out_d[0:128, 0:512], ob1[:])
                  continue

              # ------------- phase 2+3: banded attention, out-proj, RS
              with ExitStack() as p2:
                ptp = p2.enter_context(tc.tile_pool(name="ptp", bufs=5))
                sp = p2.enter_context(tc.tile_pool(name="sp", bufs=10))
                smp = p2.enter_context(tc.tile_pool(name="smp", bufs=12))
                atp = p2.enter_context(tc.tile_pool(name="atp", bufs=1))
                wop = p2.enter_context(tc.tile_pool(name="wop", bufs=1))
                obp = p2.enter_context(tc.tile_pool(name="obp", bufs=6))

                attnT = atp.tile([128, 4, S], bf16, tag="attnT")
                wo_sb = wop.tile([128, 4, EMB], bf16, tag="wo")
                if phases >= 3:
                    for h in range(4):
                        nc.sync.dma_start(wo_sb[:, h, :], woT_r[:, h, :])

                def out_proj(pp):
                    if phases < 3 or pp < 0:
                        return
                    for sti in range(2):
                        st = 2 * pp + sti
                        for nt in range(4):
                            po = psp.tile([128, 512], f32, tag="ps",
                                          name=f"po_{st}_{nt}")
                            for h in range(4):
                                nc.tensor.matmul(
                                    po[:], attnT[:, h, st * 128:(st + 1) * 128],
                                    wo_sb[:, h, nt * 512:(nt + 1) * 512],
                                    start=(h == 0), stop=(h == 3))
                            ob = obp.tile([128, 512], f32, tag="ob")
                            nc.scalar.copy(ob[:], po[:])
                            nc.sync.dma_start(
                                part_dram[st * 128:(st + 1) * 128,
                                          nt * 512:(nt + 1) * 512], ob[:])
                    if pp % 2 == 1:
                        g = pp // 2
                        if single_core:
                            nc.sync.dma_start(
                                rs_dram[g * 128:(g + 1) * 128, :],
                                part_dram[g * 512:g * 512 + 128, :])
                        else:
                            nc.gpsimd.collective_compute(
                                "ReduceScatter", ALU.add, replica_groups=RG,
                                ins=[part_dram[g * 512:(g + 1) * 512, :].opt()],
                                outs=[rs_dram[g * 128:(g + 1) * 128, :].opt()])
                        nc.sync.dma_start(out_d[g * 128:(g + 1) * 128, :],
                                          rs_dram[g * 128:(g + 1) * 128, :])

                for p in range(8):  # pairs of q-tiles
                    for hw_ in range(1):  # full wave: 4 heads x 2 qt
                        hs = (0, 1, 2, 3)
                        PTs = {}
                        pexps = {}
                        for h in hs:
                            PT = ptp.tile([128, 4, 256], f32r, tag="PT",
                                          name=f"PT_{p}_{h}")
                            nc.vector.tensor_copy(PT[:, 0, 128:256], zr[:, 0:128])
                            nc.vector.tensor_copy(PT[:, 3, 0:128], zr[:, 0:128])
                            PTs[h] = PT
                        # wave 1: all 4 score matmuls
                        for h in hs:
                            for qi in range(2):
                                qt = 2 * p + qi
                                sc_ps = psp.tile([128, 384], f32, tag="ps",
                                                 name=f"sc_{p}_{h}_{qi}")
                                nc.tensor.matmul(
                                    sc_ps[:],
                                    qT[:, h, qt * 128:(qt + 1) * 128],
                                    kT[:, h, qt * 128: qt * 128 + 384],
                                    start=True, stop=True)
                                v_idx = min(qt, 2)
                                sc = sp.tile([128, 384], f32, tag="sc",
                                             name=f"scb_{p}_{h}_{qi}")
                                nc.vector.tensor_add(out=sc[:], in0=sc_ps[:],
                                                     in1=mb_sb[:, v_idx, :])
                                pexp = sp.tile([128, 384], f32r, tag="pexp",
                                               name=f"pexp_{p}_{h}_{qi}")
                                rs = smp.tile([128, 1], f32, tag="rs")
                                nc.scalar.activation(pexp[:], sc[:], AF.Exp,
                                                     bias=0.0, scale=SCALE,
                                                     accum_out=rs[:])
                                rr = smp.tile([128, 1], f32, tag="rr")
                                nc.vector.reciprocal(rr[:], rs[:])
                                nc.vector.tensor_scalar_mul(pexp[:], pexp[:], rr[:])
                                pexps[(h, qi)] = pexp
                        # pipelined: previous pair's out-projection fills
                        # the PE while this pair's softmax chains run
                        out_proj(p - 1)
                        # wave 2: transposes + PT assembly
                        for h in hs:
                            for qi in range(2):
                                pexp = pexps[(h, qi)]
                                pt3 = psp.tile([128, 384], f32r, tag="ps",
                                               name=f"pt3_{p}_{h}_{qi}")
                                for j in range(3):
                                    nc.tensor.transpose(
                                        pt3[:, j * 128:(j + 1) * 128],
                                        pexp[:, j * 128:(j + 1) * 128],
                                        ident_sb[:])
                                nc.scalar.copy(
                                    PTs[h][:, qi:qi + 3, qi * 128:(qi + 1) * 128],
                                    pt3[:].rearrange("p (c q) -> p c q", c=3))
                        # wave 3: AV
                        for h in hs:
                            av = psp.tile([128, 256], f32, tag="ps",
                                          name=f"av_{p}_{h}")
                            for cc in range(4):
                                nc.tensor.matmul(
                                    av[:],
                                    vP[:, 2 * p + cc, h * 128:(h + 1) * 128],
                                    PTs[h][:, cc, :],
                                    start=(cc == 0), stop=(cc == 3))
                            nc.vector.tensor_copy(
                                attnT[:, h, p * 256:(p + 1) * 256], av[:])

                out_proj(7)



    nc.compile()
    return nc


def _get_nc():
    if "nc" not in _CACHE:
        _CACHE["nc"] = _build_nc()
    return _CACHE["nc"]


# ----------------------------------------------------------------- entry
def _prepare_in_maps(x, w_qkv, w_out, w_c1, w_c2):
    import ml_dtypes

    x = np.asarray(x, dtype=np.float32)
    w_qkv = np.asarray(w_qkv, dtype=np.float32)
    w_out = np.asarray(w_out, dtype=np.float32)
    w_c1 = np.asarray(w_c1, dtype=np.float32)
    w_c2 = np.asarray(w_c2, dtype=np.float32)

    window = _host_window(x, w_c1, w_c2)
    mb = _build_maskbias(window)
    cosT, sinT = _rope_tables()
    permT = np.ascontiguousarray(_perm_mat().T)  # lhsT for rot matmul
    ident = np.eye(128, dtype=np.float32)
    bf = ml_dtypes.bfloat16

    in_maps = []
    for c in range(NCORES):
        b, g = divmod(c, 4)
        rows = slice(g * 512, g * 512 + 512)
        wq = w_qkv[0 * EMB:1 * EMB][rows]
        wk = w_qkv[1 * EMB:2 * EMB][rows]
        wv = w_qkv[2 * EMB:3 * EMB][rows]
        in_maps.append({
            "xT": np.ascontiguousarray(x[b].T).astype(bf),
            "wqkT": np.ascontiguousarray(np.concatenate([wq, wk], 0).T).astype(bf),
            "wvT": np.ascontiguousarray(wv.T).astype(bf),
            "woT": np.ascontiguousarray(w_out.T[rows]).astype(bf),
            "cosT": cosT.astype(bf), "sinT": sinT.astype(bf),
            "permT": permT, "ident": ident,
            "mb": mb,
        })
    return in_maps


def _assemble(results):
    out = np.empty((B, S, EMB), dtype=np.float32)
    for c in range(NCORES):
        b, j = divmod(c, 4)
        r = results[c]["out"]  # [512, EMB]
        for g in range(4):
            out[b, g * 512 + j * 128: g * 512 + (j + 1) * 128] = \
                r[g * 128:(g + 1) * 128]
    return out


def kernel(x, w_qkv, w_out, w_c1, w_c2):
    from concourse.bass_utils import run_bass_kernel_spmd

    nc = _get_nc()
    in_maps = _prepare_in_maps(x, w_qkv, w_out, w_c1, w_c2)
    res = run_bass_kernel_spmd(nc, in_maps, core_ids=list(range(NCORES)))
    return _assemble(res.results)
